# revision 14
# baseline (speedup 1.0000x reference)
"""Trainium2 Bass kernel for JointSelfAttention (B=4,T=2048,C=1024,H=16).

Sharding: 8 cores = 4 batches (data-parallel) x 2 head-groups of 8 heads
(tensor-parallel).  Each core computes qkv for its head group, qk-RMSNorm,
RoPE, causal attention, and a partial c_proj; the host sums the two partial
projections per batch and transposes back.

Design notes (cost-model driven):
- All attention-side matmuls run in bf16 (1.0 PE cycles/row regardless of
  free-dim size); qkv stays f32r (1.0 cycles/row at free>=256).
- q/k head transposes and the attention-output transposes go through the
  DMA xbar engine (dma_start_transpose) instead of PE+copy round trips;
  destinations are kept contiguous (the xbar fast path requires it).
- attn@v is computed transposed (pt stationary, v-augmented moving, out
  free dim 65) which both halves the PE cost of that stage and lands the
  softmax denominator per-partition, so the divide is one reciprocal +
  one broadcast multiply on DVE.
- The exp element throughput of the Activation engine (~116us/core) is the
  phase-2 floor, so attention for the first query windows (and the first
  two heads of the third) is software-pipelined INTO the qkv/rope phase -
  their k/q tiles are ready after mt3/mt7/mt11 - hiding ~60us of exp under
  PE-bound qkv work.  During that overlap window attention runs one head
  at a time with single-bank score slots (PSUM is the scarce resource);
  the remaining windows run head-PAIRED so one exp instruction covers both
  heads of a PE pair.
- The causal triangle is masked additively (-240 into the score PSUM via
  an identity-stationary matmul) BEFORE exp, so exp(masked)~=0 and no
  post-exp trim sits in the score->exp->av critical path.
- softmax is max-free: q/k are RMS-normalised so |scores/8| <= 8.
"""

import math
import numpy as np
from contextlib import ExitStack

B, T, C, H, HD = 4, 2048, 1024, 16, 64
HG = 2              # head groups (tensor-parallel dim)
HPG = H // HG       # heads per group = 8
CG = HPG * HD       # channels per group = 512
N_CORES = B * HG
EPS = float(np.finfo(np.float32).eps)
QW = 512            # query window
NQW = T // QW       # 4
NMT = T // 128      # 16 token tiles
NKC = C // 128      # 8 contraction tiles for qkv
SCALE = 1.0 / math.sqrt(HD)
MASKVAL = -240.0    # raw-score additive mask; exp((s+M)/8) <= e^-22


def _split_excess_waits(nc, mybir, max_waits=1):
    """This container's walrus only encodes 1 sync-wait per instruction
    ("Too many sync wait commands" in CoreV3 codegen).  Move extra waits to
    preceding NoOps on the same engine."""
    for f in nc.m.functions:
        for bb in f.blocks:
            new_insts = []
            for inst in bb.instructions:
                si = inst.sync_info
                if si is not None and si.on_wait and len(si.on_wait) > max_waits:
                    waits = list(si.on_wait)
                    extra, keep = waits[:-max_waits], waits[-max_waits:]
                    for i in range(0, len(extra), max_waits):
                        nop = mybir.InstNoOp(
                            name=f"{inst.name}-ws{i}", ins=[], outs=[])
                        nop.engine = inst.engine
                        nop.sync_info = mybir.SyncInfo(
                            on_wait=extra[i:i + max_waits], on_update=[])
                        new_insts.append(nop)
                    inst.sync_info = mybir.SyncInfo(
                        on_wait=keep, on_update=list(si.on_update or []))
                new_insts.append(inst)
            bb.instructions.clear()
            bb.instructions.extend(new_insts)


def _build_nc():
    import concourse.bass as bass
    import concourse.tile as tile
    from concourse import mybir

    f32 = mybir.dt.float32
    f32r = mybir.dt.float32r
    bf16 = mybir.dt.bfloat16
    AF = mybir.ActivationFunctionType

    nc = bass.Bass("TRN2", debug=False, num_devices=N_CORES)

    xt = nc.dram_tensor("xt", [NMT, NKC, 128, 128], f32r, kind="ExternalInput").ap()
    wqk = nc.dram_tensor("wqk", [C, 2 * CG], bf16, kind="ExternalInput").ap()
    wv = nc.dram_tensor("wv", [C, CG], bf16, kind="ExternalInput").ap()
    wp = nc.dram_tensor("wp", [CG, C], bf16, kind="ExternalInput").ap()
    cfd = nc.dram_tensor("cfd", [128, NMT, HD], bf16, kind="ExternalInput").ap()
    spd = nc.dram_tensor("spd", [128, NMT, HD], bf16, kind="ExternalInput").ap()
    maskd = nc.dram_tensor("maskd", [128, 128], bf16, kind="ExternalInput").ap()
    identd = nc.dram_tensor("identd", [128, 128], bf16, kind="ExternalInput").ap()
    onesd = nc.dram_tensor("onesd", [128, 128], bf16, kind="ExternalInput").ap()
    out = nc.dram_tensor("o", [C, T], f32, kind="ExternalOutput").ap()

    with tile.TileContext(nc) as tc:
        with ExitStack() as ctx:
            persist = ctx.enter_context(tc.tile_pool(name="persist", bufs=1))
            wqk_sb = persist.tile([128, NKC, 2 * CG], bf16)
            wv_sb = persist.tile([128, NKC, CG], bf16)
            wp_sb = persist.tile([128, CG // 128, C], bf16)
            qT = persist.tile([128, NMT, HPG // 2, 128], bf16)  # (hd-ch x t) pair-blocked
            kT = persist.tile([128, NMT, HPG // 2, 128], bf16)
            vaug = persist.tile([128, NMT, HPG, HD + 1], bf16)  # v + ones col
            cf_sb = persist.tile([128, NMT, HD], bf16)   # [cos, cos]
            sp_sb = persist.tile([128, NMT, HD], bf16)   # [sin, -sin]
            mask_sb = persist.tile([128, 128], bf16)     # 0 / -240 causal
            ident_sb = persist.tile([128, 128], bf16)
            eps_sb = persist.tile([128, 1], f32)

            nc.vector.memset(eps_sb[:], EPS)
            nc.gpsimd.memset(vaug[:, :, :, HD:HD + 1], 1.0)
            # weights/constants chunked + ordered so mt0's matmuls unblock
            # ASAP: q-weight chunks first, then rope tables, v/k weights,
            # attention constants, and c_proj weights last.
            wqk_r = wqk.rearrange("(kc p) n -> p kc n", p=128)
            wv_r = wv.rearrange("(kc p) n -> p kc n", p=128)
            for kc in range(NKC):
                nc.scalar.dma_start(wqk_sb[:, kc], wqk_r[:, kc])
            nc.scalar.dma_start(cf_sb[:], cfd[:])
            nc.scalar.dma_start(sp_sb[:], spd[:])
            for kc in range(NKC):
                nc.scalar.dma_start(wv_sb[:, kc], wv_r[:, kc])
            nc.scalar.dma_start(mask_sb[:], maskd[:])
            nc.scalar.dma_start(ident_sb[:], identd[:])
            nc.scalar.dma_start(wp_sb[:], wp.rearrange("(kc p) n -> p kc n", p=128))

            # shared phase-2 SBUF pools (used by both overlap + tail segments)
            ytp = ctx.enter_context(tc.tile_pool(name="ytp", bufs=2))
            ybq = ctx.enter_context(tc.tile_pool(name="ybq", bufs=2))
            rdp = ctx.enter_context(tc.tile_pool(name="rdp", bufs=2))
            osp = ctx.enter_context(tc.tile_pool(name="osp", bufs=3))
            ytbs = {qw: ytp.tile([128, 4, HPG, HD], bf16, tag="ytb",
                                 name=f"ytb_{qw}")
                    for qw in range(NQW)}
            ybts = {}

            def qw_transpose(qw):
                # y back to (channel x token) via the DMA xbar
                ybt = ybq.tile([128, 4, HPG // 2, 128], bf16,
                               tag="ybt", name=f"ybt_{qw}")
                ytb = ytbs[qw]
                for j in range(4):
                    nc.sync.dma_start_transpose(ybt[:, j], ytb[:, j])
                ybts[qw] = ybt

            def qw_proj(qw, psum_pool, psum_tag):
                ybt = ybts[qw]
                for mo in range(C // 128):
                    op_ps = psum_pool.tile([128, QW], f32, tag=psum_tag,
                                           name=f"op_{qw}_{mo}")
                    for kc in range(CG // 128):
                        nc.tensor.matmul(
                            op_ps[:],
                            wp_sb[:, kc, mo * 128:(mo + 1) * 128],
                            bass.AP(tensor=ybt.tensor,
                                    offset=ybt.offset + kc * 128,
                                    ap=[ybt.ap[0], [4 * 128, 4], [1, 128]]),
                            start=(kc == 0), stop=(kc == CG // 128 - 1))
                    ot = osp.tile([128, QW], f32, tag="ot",
                                  name=f"ot_{qw}_{mo}")
                    nc.vector.tensor_copy(ot[:], op_ps[:])
                    nc.sync.dma_start(
                        out[mo * 128:(mo + 1) * 128, qw * QW:(qw + 1) * QW],
                        ot[:])

            def epilogue(y_t, qw, h):
                rd = rdp.tile([128, 4], f32, tag="rd", name=f"rd_{qw}_{h}")
                nc.vector.reciprocal(rd[:], y_t[:, :, HD])
                rd_b = bass.AP(tensor=rd.tensor, offset=rd.offset,
                               ap=[rd.ap[0], [1, 4], [0, HD]])
                nc.vector.tensor_tensor(
                    ytbs[qw][:, :, h, :], y_t[:, :, 0:HD], rd_b,
                    op=mybir.AluOpType.mult)

            def score_mm(sc_slice, h, qw, kt, col0):
                po, tr = (h % 2) * 64, h // 2
                nc.tensor.matmul(
                    sc_slice,
                    kT[po:po + 64, kt, tr, :],
                    qT[po:po + 64, 4 * qw + col0 // 128:4 * (qw + 1), tr, :],
                    start=True, stop=(col0 == 0))

            def mask_mm(sc_block):
                nc.tensor.matmul(sc_block, ident_sb[:], mask_sb[:],
                                 start=False, stop=True)

            def qw_tiles(qw):
                return [(kt, 0) for kt in range(4 * qw)] + \
                       [(4 * qw + d, d * 128) for d in range(4)]

            # ---- segment A+B: qkv/norm/rope for all mts, with early-window
            # attention interleaved once its k/q tiles are ready ----
            with ExitStack() as p1:
                xp = p1.enter_context(tc.tile_pool(name="xp", bufs=3))
                qkps = p1.enter_context(tc.tile_pool(name="qkps", bufs=3, space="PSUM"))
                vps = p1.enter_context(tc.tile_pool(name="vps", bufs=1, space="PSUM"))
                sqp = p1.enter_context(tc.tile_pool(name="sqp", bufs=2))
                stp = p1.enter_context(tc.tile_pool(name="stp", bufs=2))
                rp = p1.enter_context(tc.tile_pool(name="rp", bufs=2))
                sco = p1.enter_context(tc.tile_pool(name="sco", bufs=2, space="PSUM"))
                yo = p1.enter_context(tc.tile_pool(name="yo", bufs=2, space="PSUM"))
                pto = p1.enter_context(tc.tile_pool(name="pto", bufs=3))

                def mt_chunks(mt):
                    st = {}

                    def c0():
                        st["xt"] = xp.tile([128, NKC, 128], f32r, tag="xt",
                                           name=f"xt_{mt}")
                        nc.sync.dma_start(st["xt"][:], xt[mt])
                        st["q"] = qkps.tile([128, CG], f32, tag="qk",
                                            name=f"q_{mt}")
                        for kc in range(4):
                            nc.tensor.matmul(
                                st["q"][:], st["xt"][:, kc, :],
                                wqk_sb[:, kc, 0:CG],
                                start=(kc == 0), stop=False)

                    def c1():
                        for kc in range(4, NKC):
                            nc.tensor.matmul(
                                st["q"][:], st["xt"][:, kc, :],
                                wqk_sb[:, kc, 0:CG],
                                start=False, stop=(kc == NKC - 1))

                    def c2():
                        st["k"] = qkps.tile([128, CG], f32, tag="qk",
                                            name=f"k_{mt}")
                        for kc in range(4):
                            nc.tensor.matmul(
                                st["k"][:], st["xt"][:, kc, :],
                                wqk_sb[:, kc, CG:2 * CG],
                                start=(kc == 0), stop=False)

                    def c3():
                        for kc in range(4, NKC):
                            nc.tensor.matmul(
                                st["k"][:], st["xt"][:, kc, :],
                                wqk_sb[:, kc, CG:2 * CG],
                                start=False, stop=(kc == NKC - 1))

                    def c4():
                        st["v"] = vps.tile([128, CG], f32, tag="v",
                                           name=f"v_{mt}")
                        for kc in range(NKC):
                            nc.tensor.matmul(
                                st["v"][:], st["xt"][:, kc, :], wv_sb[:, kc, :],
                                start=(kc == 0), stop=(kc == NKC - 1))

                    def c5():
                        nc.scalar.copy(
                            vaug[:, mt, :, 0:HD],
                            st["v"][:].rearrange("p (h d) -> p h d", d=HD))
                        sq = sqp.tile([128, 2 * CG], f32, tag="sq",
                                      name=f"sq_{mt}")
                        nc.scalar.activation(sq[:, 0:CG], st["q"][:], AF.Square)
                        nc.scalar.activation(sq[:, CG:2 * CG], st["k"][:], AF.Square)
                        ss = stp.tile([128, 2 * HPG], f32, tag="ss",
                                      name=f"ss_{mt}")
                        nc.vector.tensor_reduce(
                            ss[:], sq[:].rearrange("p (h d) -> p h d", d=HD),
                            axis=mybir.AxisListType.X, op=mybir.AluOpType.add)
                        st["ss"] = ss

                    def c6():
                        rms = stp.tile([128, 2 * HPG], f32, tag="rms",
                                       name=f"rms_{mt}")
                        nc.scalar.activation(rms[:], st["ss"][:], AF.Sqrt,
                                             bias=eps_sb[:], scale=1.0 / HD)
                        rr = stp.tile([128, 2 * HPG], f32, tag="rr",
                                      name=f"rr_{mt}")
                        nc.vector.reciprocal(rr[:], rms[:])
                        qkn = rp.tile([128, 2 * HPG, HD], bf16, tag="qkn",
                                      name=f"qkn_{mt}")
                        rr_bq = bass.AP(tensor=rr.tensor, offset=rr.offset,
                                        ap=[rr.ap[0], [1, HPG], [0, HD]])
                        rr_bk = bass.AP(tensor=rr.tensor, offset=rr.offset + HPG,
                                        ap=[rr.ap[0], [1, HPG], [0, HD]])
                        nc.vector.tensor_tensor(
                            qkn[:, 0:HPG, :],
                            st["q"][:].rearrange("p (h d) -> p h d", d=HD),
                            rr_bq, op=mybir.AluOpType.mult)
                        nc.vector.tensor_tensor(
                            qkn[:, HPG:2 * HPG, :],
                            st["k"][:].rearrange("p (h d) -> p h d", d=HD),
                            rr_bk, op=mybir.AluOpType.mult)
                        st["qkn"] = qkn

                    def c7():
                        qkn = st["qkn"]
                        qr = rp.tile([128, 2 * HPG, HD], bf16, tag="qr",
                                     name=f"qr_{mt}")
                        cf_b = bass.AP(tensor=cf_sb.tensor,
                                       offset=cf_sb.offset + mt * HD,
                                       ap=[cf_sb.ap[0], [0, 2 * HPG], [1, HD]])
                        nc.vector.tensor_tensor(qr[:], qkn[:], cf_b,
                                                op=mybir.AluOpType.mult)
                        ts_t = rp.tile([128, 2 * HPG, HD], bf16, tag="ts",
                                       name=f"ts_{mt}")
                        sp_lo = bass.AP(tensor=sp_sb.tensor,
                                        offset=sp_sb.offset + mt * HD,
                                        ap=[sp_sb.ap[0], [0, 2 * HPG], [1, HD // 2]])
                        sp_hi = bass.AP(tensor=sp_sb.tensor,
                                        offset=sp_sb.offset + mt * HD + HD // 2,
                                        ap=[sp_sb.ap[0], [0, 2 * HPG], [1, HD // 2]])
                        nc.vector.tensor_tensor(
                            ts_t[:, :, 0:HD // 2], qkn[:, :, HD // 2:HD], sp_lo,
                            op=mybir.AluOpType.mult)
                        nc.vector.tensor_tensor(
                            ts_t[:, :, HD // 2:HD], qkn[:, :, 0:HD // 2], sp_hi,
                            op=mybir.AluOpType.mult)
                        nc.vector.tensor_add(qr[:], qr[:], ts_t[:])
                        nc.sync.dma_start_transpose(qT[:, mt], qr[:, 0:HPG, :])
                        nc.sync.dma_start_transpose(kT[:, mt],
                                                    qr[:, HPG:2 * HPG, :])

                    return [c0, c1, c2, c3, c4, c5, c6, c7]

                def ovl_units(qw, heads):
                    """Single-head attention units for the overlap window."""
                    tiles = qw_tiles(qw)
                    nvalid = [sum(1 for (kt, c0) in tiles if c0 <= 128 * j)
                              for j in range(4)]
                    for h in heads:
                        y_t = yo.tile([128, 4, HD + 1], f32, tag="yo",
                                      name=f"yo_{qw}_{h}")
                        seen = [0] * 4
                        for ti, (kt, col0) in enumerate(tiles):
                            st = {}

                            def pre(kt=kt, col0=col0, h=h, ti=ti, st=st):
                                sc = sco.tile([128, QW], f32, tag="sco",
                                              name=f"sco_{qw}_{h}_{ti}")
                                score_mm(sc[:, col0:], h, qw, kt, col0)
                                if col0 > 0:
                                    mask_mm(sc[:, col0:col0 + 128])
                                st["sc"] = sc

                            def post(kt=kt, col0=col0, h=h, ti=ti, st=st,
                                     y_t=y_t, seen=seen,
                                     last=(ti == len(tiles) - 1)):
                                sc = st["sc"]
                                pt = pto.tile([128, QW], bf16, tag="pto",
                                              name=f"pto_{qw}_{h}_{ti}")
                                nc.scalar.activation(pt[:, col0:],
                                                     sc[:, col0:],
                                                     AF.Exp, scale=SCALE)
                                for j in range(4):
                                    if col0 <= 128 * j:
                                        seen[j] += 1
                                        nc.tensor.matmul(
                                            y_t[:, j, :],
                                            pt[:, 128 * j:128 * (j + 1)],
                                            vaug[:, kt, h, :],
                                            start=(seen[j] == 1),
                                            stop=(seen[j] == nvalid[j]))
                                if last:
                                    epilogue(y_t, qw, h)

                            yield pre, post

                # Emission schedule: mts 0-3 plain; mts 4-7 carry qw0
                # (32 units); mts 8-15 carry qw1 + qw2-heads-0/1 (88 units,
                # qw2 units only enter the queue after mt11); tails for qw0
                # land between mt9/mt10.
                from collections import deque
                queue = deque()
                pending = None

                def run_units(budget):
                    nonlocal pending
                    for _ in range(budget):
                        if not queue:
                            return
                        pre, post = queue.popleft()
                        pre()
                        if pending is not None:
                            pending()
                        pending = post

                for mt in range(NMT):
                    if mt == 4:
                        queue.extend(ovl_units(0, range(HPG)))
                    if mt == 8:
                        queue.extend(ovl_units(1, range(HPG)))
                    if mt == 12:
                        queue.extend(ovl_units(2, (0, 1)))
                    chunks = mt_chunks(mt)
                    mts_left = NMT - mt
                    per_chunk = 2 if len(queue) > 8 * (mts_left - 1) else 1
                    for chunk in chunks:
                        run_units(per_chunk)
                        chunk()
                    if mt == 9:
                        qw_transpose(0)
                    if mt == 10:
                        if pending is not None:
                            pending()
                            pending = None
                        qw_proj(0, sco, "sco")
                while queue:
                    run_units(1)
                if pending is not None:
                    pending()
                    pending = None

            # ---- segment C: rest of qw2 + qw3, head-PAIRED exp ----
            with ExitStack() as p2:
                scp = p2.enter_context(tc.tile_pool(name="scp", bufs=3, space="PSUM"))
                yp = p2.enter_context(tc.tile_pool(name="yp", bufs=2, space="PSUM"))
                ptp = p2.enter_context(tc.tile_pool(name="ptp", bufs=3))

                def attn_units(qw, tr):
                    tiles = qw_tiles(qw)
                    y_tiles = [yp.tile([128, 4, HD + 1], f32, tag="y",
                                       name=f"y_{qw}_{tr}_{hi}")
                               for hi in range(2)]
                    nvalid = [sum(1 for (kt, c0) in tiles if c0 <= 128 * j)
                              for j in range(4)]
                    seen = [[0] * 4, [0] * 4]
                    state = {}

                    def pre(kt, col0, ti):
                        sc = scp.tile([128, 2, QW], f32, tag="sc",
                                      name=f"sc_{qw}_{tr}_{ti}")
                        for hi in range(2):
                            score_mm(sc[:, hi, col0:], 2 * tr + hi, qw, kt, col0)
                            if col0 > 0:
                                mask_mm(sc[:, hi, col0:col0 + 128])
                        state[ti] = sc

                    def post(kt, col0, ti, last):
                        sc = state.pop(ti)
                        pt = ptp.tile([128, 2, QW], bf16, tag="pt",
                                      name=f"pt_{qw}_{tr}_{ti}")
                        nc.scalar.activation(pt[:, :, col0:], sc[:, :, col0:],
                                             AF.Exp, scale=SCALE)
                        for j in range(4):
                            if col0 <= 128 * j:
                                for hi in range(2):
                                    seen[hi][j] += 1
                                    nc.tensor.matmul(
                                        y_tiles[hi][:, j, :],
                                        pt[:, hi, 128 * j:128 * (j + 1)],
                                        vaug[:, kt, 2 * tr + hi, :],
                                        start=(seen[hi][j] == 1),
                                        stop=(seen[hi][j] == nvalid[j]))
                        if last:
                            for hi in range(2):
                                epilogue(y_tiles[hi], qw, 2 * tr + hi)

                    n = len(tiles)
                    for ti, (kt, col0) in enumerate(tiles):
                        yield (lambda kt=kt, col0=col0, ti=ti: pre(kt, col0, ti),
                               lambda kt=kt, col0=col0, ti=ti, last=(ti == n - 1):
                               post(kt, col0, ti, last))

                units = []
                for qw, trs in ((2, (1, 2, 3)), (3, (0, 1, 2, 3))):
                    for tr in trs:
                        for ti, (pre, post) in enumerate(attn_units(qw, tr)):
                            units.append((qw, tr, ti, pre, post))

                from collections import deque as _dq
                pending2 = _dq()
                done = set()

                def tails(qw, tr, ti):
                    if qw == 2 and tr == 1 and ti == 1 and "t1" not in done:
                        qw_transpose(1)
                        done.add("t1")
                    if qw == 2 and tr == 2 and ti == 1 and "p1" not in done:
                        qw_proj(1, scp, "sc")
                        done.add("p1")
                    if qw == 3 and tr == 0 and ti == 3 and "t2" not in done:
                        qw_transpose(2)
                        done.add("t2")
                    if qw == 3 and tr == 1 and ti == 3 and "p2" not in done:
                        qw_proj(2, scp, "sc")
                        done.add("p2")

                for (qw, tr, ti, pre, post) in units:
                    pre()
                    tails(qw, tr, ti)
                    while len(pending2) >= 2:
                        pending2.popleft()()
                    pending2.append(post)
                while pending2:
                    pending2.popleft()()
                qw_transpose(3)
                qw_proj(3, scp, "sc")

    _split_excess_waits(nc, mybir)
    return nc


_NC_CACHE = {}


def _get_nc():
    if "nc" not in _NC_CACHE:
        _NC_CACHE["nc"] = _build_nc()
    return _NC_CACHE["nc"]


def _host_inputs(x, w_attn, w_proj):
    import ml_dtypes
    bf = ml_dtypes.bfloat16
    inv_freq = 1.0 / (10000.0 ** (np.arange(0, HD, 2, dtype=np.float32) / HD))
    t = np.arange(T, dtype=np.float32)
    freqs = np.outer(t, inv_freq)  # (T, 32)
    cos = np.cos(freqs).astype(bf).astype(np.float32)
    sin = np.sin(freqs).astype(bf).astype(np.float32)
    cf = np.concatenate([cos, cos], axis=1)      # (T, 64)
    sp = np.concatenate([sin, -sin], axis=1)     # (T, 64)
    cfd = np.ascontiguousarray(
        cf.reshape(NMT, 128, HD).transpose(1, 0, 2)).astype(bf)
    spd = np.ascontiguousarray(
        sp.reshape(NMT, 128, HD).transpose(1, 0, 2)).astype(bf)
    kl = np.arange(128)[:, None]
    ql = np.arange(128)[None, :]
    mask = np.where(kl > ql, MASKVAL, 0.0).astype(np.float32).astype(bf)
    ident = np.eye(128, dtype=np.float32).astype(bf)
    onesb = np.ones((128, 128), dtype=np.float32).astype(bf)

    in_maps = []
    for b in range(B):
        xT = np.ascontiguousarray(x[b].T)  # (C, T)
        xt = np.ascontiguousarray(
            xT.reshape(NKC, 128, NMT, 128).transpose(2, 0, 1, 3))
        for hg in range(HG):
            qr = slice(hg * CG, (hg + 1) * CG)
            kr = slice(C + hg * CG, C + (hg + 1) * CG)
            vr = slice(2 * C + hg * CG, 2 * C + (hg + 1) * CG)
            wqk = np.ascontiguousarray(
                np.concatenate([w_attn[qr], w_attn[kr]], axis=0).T).astype(bf)
            wv = np.ascontiguousarray(w_attn[vr].T).astype(bf)
            wp = np.ascontiguousarray(
                w_proj[:, hg * CG:(hg + 1) * CG].T).astype(bf)
            in_maps.append({
                "xt": xt, "wqk": wqk, "wv": wv, "wp": wp,
                "cfd": cfd, "spd": spd, "maskd": mask, "identd": ident,
                "onesd": onesb,
            })
    return in_maps


def kernel(x, w_attn, w_proj, _profile=False):
    from concourse.bass_utils import run_bass_kernel_spmd
    nc = _get_nc()
    in_maps = _host_inputs(
        np.asarray(x, dtype=np.float32),
        np.asarray(w_attn, dtype=np.float32),
        np.asarray(w_proj, dtype=np.float32))
    res = run_bass_kernel_spmd(nc, in_maps, core_ids=list(range(N_CORES)),
                               trace=_profile)
    out = np.empty((B, T, C), dtype=np.float32)
    for b in range(B):
        acc = res.results[2 * b]["o"] + res.results[2 * b + 1]["o"]
        out[b] = acc.T
    if _profile:
        return out, res
    return out
